# revision 15
# baseline (speedup 1.0000x reference)
"""Multi-head causal attention (B=2, S=2048, D=2048, H=16) on 8 trn2 cores.

Sharding: tensor-parallel over heads. Core c owns heads 2c, 2c+1 (256
features of q/k/v). Each core computes its heads' QKV projections (+RoPE),
causal attention, and a partial output through its slice of wo. The 8
partial outputs are summed on the host (the "all-reduce").

All matmul operands are bf16 (fp32 PSUM accumulation) — bf16 enables the
PE fast-weight-load path and 1 col/cycle streaming; fp32 inputs are
rounded on the host. Output partials are written fp16 and summed in fp32
on the host.

Layouts on device (per core):
  qT, kT: [hd=128 partitions, head, token] bf16 (features on partitions)
          head dims permuted (evens then odds) via host-permuted wq/wk rows
          so RoPE pairs sit in partition halves.
  v:      [token, feature] bf16, computed directly with x subtiles as the
          stationary operand (lhsT) and wv as the moving operand — no PE
          transposes needed.
  scoresT chunk = matmul(lhsT=kT chunk, rhs=qT tile) -> [kt 128, q 512] PSUM
  probsT = exp(scoresT/sqrt(hd)) bf16 (no max subtraction: scores O(1))
  attnT accum = matmul(lhsT=v chunk, rhs=probsT) -> [hd, q] PSUM
  denom = matmul(lhsT=ones[128,128], rhs=probsT) -> [128, q] PSUM
          (every partition holds the denom row: broadcast for free)
  recip = reciprocal_approx_fast(denom) -> [128, q] f32
  aT    = a_ps * recip (DVE) -> bf16
  out rows = matmul(lhsT=aT t-sub, rhs=woT) -> [t 128, j] PSUM -> fp16 -> DRAM.

RoPE (per psum eviction tile qp [128, 512], top rows = even dims xr,
bottom = odd dims xi):
  qswap = [xi; xr]        (2 ACT half-copies)
  p1 = qp * [c; c]        (DVE, psum x sbuf)
  p2 = qswap * [s; -s]    (DVE)
  qT  = p1 - p2 = [xr*c - xi*s ; xi*c + xr*s]   (DVE -> bf16)
"""

import math

import numpy as np

B = 2
S = 2048
D = 2048
H = 16
HD = 128
NCORES = 8
FPC = D // NCORES          # 256 features (2 heads) per core
P = 128
ND = D // P                # 16 contraction chunks
TT_N = 512                 # token tile (matmul free dim)
NTT = S // TT_N            # 4 token tiles per batch
NKT = S // P               # 16 key chunks per batch
SCALE = 1.0 / math.sqrt(HD)

_CACHE = {}


def _build_nc():
    import concourse.bass as bass  # noqa: F401
    from concourse import bacc
    import concourse.mybir as mybir
    import concourse.tile as tile

    f32 = mybir.dt.float32
    bf16 = mybir.dt.bfloat16
    f16 = mybir.dt.float16
    MUL = mybir.AluOpType.mult
    SUB = mybir.AluOpType.subtract

    nc = bacc.Bacc(None, target_bir_lowering=False)

    xTp = nc.dram_tensor("xTp", [P, ND, B * S], bf16, kind="ExternalInput")
    wqp = nc.dram_tensor("wqp", [P, ND, FPC], bf16, kind="ExternalInput")
    wkp = nc.dram_tensor("wkp", [P, ND, FPC], bf16, kind="ExternalInput")
    wvp = nc.dram_tensor("wvp", [P, ND, FPC], bf16, kind="ExternalInput")
    wop = nc.dram_tensor("wop", [P, 2, D], bf16, kind="ExternalInput")
    cosS = nc.dram_tensor("cosS", [P, S], bf16, kind="ExternalInput")
    sinS = nc.dram_tensor("sinS", [P, S], bf16, kind="ExternalInput")  # [s; -s]
    masks = nc.dram_tensor("masks", [P, P], bf16, kind="ExternalInput")
    onesd = nc.dram_tensor("onesd", [P, P], bf16, kind="ExternalInput")
    outp = nc.dram_tensor("outp", [B * S, D], f16, kind="ExternalOutput")

    with tile.TileContext(nc) as tc:
        with (
            tc.tile_pool(name="res", bufs=1) as res,
            tc.tile_pool(name="xp", bufs=3) as xp,
            tc.tile_pool(name="aTp", bufs=2) as aTp,
            tc.tile_pool(name="probsp", bufs=10) as probsp,
            tc.tile_pool(name="ropep", bufs=2) as ropep,
            tc.tile_pool(name="recipp", bufs=2) as recipp,
            tc.tile_pool(name="ostp", bufs=4) as ostp,
            tc.tile_pool(name="ps_big", bufs=4, space="PSUM") as ps_big,
            tc.tile_pool(name="ps_av", bufs=2, space="PSUM") as ps_av,
            tc.tile_pool(name="ps_d", bufs=2, space="PSUM") as ps_d,
        ):
            # resident tensors
            wq_sb = res.tile([P, ND, FPC], bf16)
            wk_sb = res.tile([P, ND, FPC], bf16)
            wv_sb = res.tile([P, ND, FPC], bf16)
            wo_sb = res.tile([P, 2, D], bf16)
            mask_sb = res.tile([P, P], bf16)
            ones_sb = res.tile([P, P], bf16)
            qT_sbs = [res.tile([P, 2, S], bf16, name=f"qT{i}")
                      for i in range(2)]
            kT_sbs = [res.tile([P, 2, S], bf16, name=f"kT{i}")
                      for i in range(2)]
            v_sbs = [res.tile([P, NKT, FPC], bf16, name=f"v{i}")
                     for i in range(2)]
            cos_sb = res.tile([P, S], bf16)
            sin_sb = res.tile([P, S], bf16)

            # weights/tables: critical loads first (wq/wk chunk-0 halves gate
            # the first matmul; wv gates the v loop; cos/sin gate RoPE; wo
            # and ones aren't needed until attention/wo jobs)
            nc.scalar.dma_start(out=wq_sb[:, 0:8, :], in_=wqp[:, 0:8, :])
            nc.scalar.dma_start(out=wk_sb[:, 0:8, :], in_=wkp[:, 0:8, :])
            nc.scalar.dma_start(out=wq_sb[:, 8:16, :], in_=wqp[:, 8:16, :])
            nc.scalar.dma_start(out=wk_sb[:, 8:16, :], in_=wkp[:, 8:16, :])
            wo_jobs = []

            def emit_wo_job(trow0, aT, ts):
                trow = trow0 + ts * P
                ost = ostp.tile([P, D], f16, name="ost")
                for jc in range(D // TT_N):
                    o_ps = ps_big.tile([P, TT_N], f32, name="big")
                    for h in range(2):
                        nc.tensor.matmul(
                            o_ps[:],
                            aT[:, h, ts * P:(ts + 1) * P],
                            wo_sb[:, h, jc * TT_N:(jc + 1) * TT_N],
                            start=(h == 0), stop=(h == 1),
                        )
                    osl = slice(jc * TT_N, (jc + 1) * TT_N)
                    if jc % 2 == 0:
                        nc.vector.tensor_copy(ost[:, osl], o_ps[:])
                    else:
                        nc.scalar.copy(ost[:, osl], o_ps[:])
                    if jc == 1:
                        nc.sync.dma_start(
                            out=outp[trow:trow + P, 0:2 * TT_N],
                            in_=ost[:, 0:2 * TT_N])
                nc.sync.dma_start(out=outp[trow:trow + P, 2 * TT_N:D],
                                  in_=ost[:, 2 * TT_N:D])

            def emit_qkv(b, tt):
                qT_sb = qT_sbs[b % 2]
                kT_sb = kT_sbs[b % 2]
                v_sb = v_sbs[b % 2]
                t0g = b * S
                tsl = slice(tt * TT_N, (tt + 1) * TT_N)
                gsl = slice(t0g + tt * TT_N, t0g + (tt + 1) * TT_N)

                xt = xp.tile([P, ND, TT_N], bf16, name="xt")
                for q4 in range(4):
                    nc.sync.dma_start(
                        out=xt[:, q4 * 4:(q4 + 1) * 4, :],
                        in_=xTp[:, q4 * 4:(q4 + 1) * 4, gsl])
                if b == 0 and tt == 0:
                    nc.sync.dma_start(out=wv_sb[:, 0:8, :], in_=wvp[:, 0:8, :])
                    nc.sync.dma_start(out=wv_sb[:, 8:16, :],
                                      in_=wvp[:, 8:16, :])
                    nc.sync.dma_start(out=cos_sb[:], in_=cosS[:])
                    nc.sync.dma_start(out=sin_sb[:], in_=sinS[:])
                    nc.sync.dma_start(out=mask_sb[:], in_=masks[:])

                qk_ps = [ps_big.tile([P, TT_N], f32, name="big")
                         for _ in range(4)]

                for d in range(ND):
                    for fc in range(2):
                        nc.tensor.matmul(
                            qk_ps[fc][:],
                            wq_sb[:, d, fc * P:(fc + 1) * P],
                            xt[:, d, :],
                            start=(d == 0), stop=(d == ND - 1),
                        )
                        nc.tensor.matmul(
                            qk_ps[2 + fc][:],
                            wk_sb[:, d, fc * P:(fc + 1) * P],
                            xt[:, d, :],
                            start=(d == 0), stop=(d == ND - 1),
                        )

                # RoPE + eviction for q,k (p2 on the idle gpsimd engine
                # keeps the DVE queue short so PSUM frees promptly)
                for i, dst in ((0, qT_sb), (1, qT_sb), (2, kT_sb), (3, kT_sb)):
                    fc = i % 2
                    qp = qk_ps[i]
                    qsw = ropep.tile([P, TT_N], f32, name="qsw")
                    nc.scalar.copy(qsw[0:64, :], qp[64:128, :])
                    nc.scalar.copy(qsw[64:128, :], qp[0:64, :])
                    p1 = ropep.tile([P, TT_N], f32, name="p1")
                    nc.vector.tensor_tensor(
                        out=p1[:], in0=qp[:], in1=cos_sb[:, tsl], op=MUL)
                    p2 = ropep.tile([P, TT_N], f32, name="p2")
                    nc.gpsimd.tensor_tensor(
                        out=p2[:], in0=qsw[:], in1=sin_sb[:, tsl], op=MUL)
                    nc.vector.tensor_tensor(
                        out=dst[:, fc, tsl], in0=p1[:], in1=p2[:], op=SUB)

                # v: [t, f] blocks, one rotating PSUM tile per t-subtile
                for sub in range(4):
                    v_ps = ps_av.tile([P, FPC], f32, name="av")
                    for d in range(ND):
                        nc.tensor.matmul(
                            v_ps[:],
                            xt[:, d, sub * P:(sub + 1) * P],
                            wv_sb[:, d, :],
                            start=(d == 0), stop=(d == ND - 1),
                        )
                    nc.scalar.copy(v_sb[:, tt * 4 + sub, :], v_ps[:])

            def emit_attention(b, qt):
                qT_sb = qT_sbs[b % 2]
                kT_sb = kT_sbs[b % 2]
                v_sb = v_sbs[b % 2]
                t0g = b * S
                nkt = 4 * qt + 4
                aT = aTp.tile([P, 2, TT_N], bf16, name="aT")
                niter = 2 * (nkt + 3)
                njobs = len(wo_jobs)
                drain_every = max(1, niter // (njobs + 1)) if njobs else 0
                it = 0
                LAG = 3
                for h in range(2):
                    a_ps = ps_av.tile([P, TT_N], f32, name="av")
                    d_ps = ps_d.tile([P, TT_N], f32, name="dp")
                    pend = {}
                    for kt in range(nkt + LAG):
                        it += 1
                        if wo_jobs and drain_every and it % drain_every == 0:
                            emit_wo_job(*wo_jobs.pop(0))
                        if kt < nkt:
                            o = kt - 4 * qt
                            c0 = max(o, 0) * P
                            csl = slice(c0, TT_N)
                            s_ps = ps_big.tile([P, TT_N], f32, name="big")
                            nc.tensor.matmul(
                                s_ps[:, csl],
                                kT_sb[:, h, kt * P:(kt + 1) * P],
                                qT_sb[:, h, qt * TT_N + c0:(qt + 1) * TT_N],
                                start=True, stop=True,
                            )
                            pr = probsp.tile([P, TT_N], bf16, name="probs")
                            nc.scalar.activation(
                                pr[:, csl], s_ps[:, csl],
                                mybir.ActivationFunctionType.Exp,
                                scale=SCALE,
                            )
                            if o >= 0:
                                nc.vector.tensor_tensor(
                                    out=pr[:, c0:c0 + P],
                                    in0=pr[:, c0:c0 + P],
                                    in1=mask_sb[:],
                                    op=MUL,
                                )
                            pend[kt] = (pr, c0)
                        j = kt - LAG
                        if j >= 0:
                            pr, c0p = pend.pop(j)
                            psl = slice(c0p, TT_N)
                            nc.tensor.matmul(
                                a_ps[:, psl],
                                v_sb[:, j, h * P:(h + 1) * P],
                                pr[:, psl],
                                start=(j == 0), stop=(j == nkt - 1),
                            )
                            nc.tensor.matmul(
                                d_ps[:, psl],
                                ones_sb[:],
                                pr[:, psl],
                                start=(j == 0), stop=(j == nkt - 1),
                            )
                            if j == nkt - 1:
                                rd = recipp.tile([P, TT_N], f32, name="rd")
                                nc.vector.reciprocal_approx_fast(
                                    out=rd[:], in_=d_ps[:])
                                nc.vector.tensor_tensor(
                                    out=aT[:, h, :], in0=a_ps[:],
                                    in1=rd[:], op=MUL)
                for ts in range(TT_N // P):
                    wo_jobs.append((t0g + qt * TT_N, aT, ts))

            # schedule: attention lags QKV by one tile, pipelined across the
            # batch boundary (batch b+1's first QKV is independent of batch
            # b's last attention)
            for b in range(B):
                emit_qkv(b, 0)
                if b == 0:
                    nc.gpsimd.dma_start(out=wo_sb[:], in_=wop[:])
                    nc.gpsimd.dma_start(out=ones_sb[:], in_=onesd[:])
                if b > 0:
                    emit_attention(b - 1, NTT - 1)
                for tt in range(1, NTT):
                    emit_qkv(b, tt)
                    emit_attention(b, tt - 1)
            emit_attention(B - 1, NTT - 1)
            while wo_jobs:
                emit_wo_job(*wo_jobs.pop(0))
    nc.compile()
    return nc


def _host_prep(x, wq, wk, wv, wo):
    import ml_dtypes

    bf16 = ml_dtypes.bfloat16

    x = np.asarray(x, dtype=np.float32)
    wq = np.asarray(wq, dtype=np.float32)
    wk = np.asarray(wk, dtype=np.float32)
    wv = np.asarray(wv, dtype=np.float32)
    wo = np.asarray(wo, dtype=np.float32)

    # x packed as [p, d, t]: xTp[p, d, t] = x[t, d*128+p]
    xTp = np.ascontiguousarray(
        x.reshape(B * S, ND, P).transpose(2, 1, 0)).astype(bf16)

    # permute q/k head dims: per head, even dims then odd dims
    perm = np.concatenate(
        [h * HD + np.concatenate([np.arange(0, HD, 2), np.arange(1, HD, 2)])
         for h in range(H)]
    )
    wq_p = wq[perm]
    wk_p = wk[perm]

    # rope tables; cos stacked twice, sin stacked [s; -s]
    inv_freq = 1.0 / (10000.0 ** (np.arange(0, HD, 2, dtype=np.float64) / HD))
    t = np.arange(S, dtype=np.float64)
    freqs = t[:, None] * inv_freq[None, :]            # [S, 64]
    cosT = np.cos(freqs).T.astype(np.float32)         # [64, S]
    sinT = np.sin(freqs).T.astype(np.float32)
    cosS = np.ascontiguousarray(np.vstack([cosT, cosT])).astype(bf16)
    sinS = np.ascontiguousarray(np.vstack([sinT, -sinT])).astype(bf16)

    # triangular causal mask for the diagonal 128x128 block
    pidx = np.arange(P)[:, None]
    qidx = np.arange(P)[None, :]
    m = np.ascontiguousarray((qidx >= pidx).astype(bf16))

    ones = np.ones((P, P), dtype=bf16)

    def pack_w(wT):  # [D, FPC] -> [128, ND, FPC]
        return np.ascontiguousarray(
            wT.reshape(ND, P, FPC).transpose(1, 0, 2)).astype(bf16)

    in_maps = []
    for c in range(NCORES):
        fs = slice(c * FPC, (c + 1) * FPC)
        woT = np.ascontiguousarray(wo[:, fs].T)        # [256, D]
        wop = np.ascontiguousarray(
            woT.reshape(2, P, D).transpose(1, 0, 2)).astype(bf16)
        in_maps.append({
            "xTp": xTp,
            "wqp": pack_w(np.ascontiguousarray(wq_p[fs].T)),
            "wkp": pack_w(np.ascontiguousarray(wk_p[fs].T)),
            "wvp": pack_w(np.ascontiguousarray(wv[fs].T)),
            "wop": wop,
            "cosS": cosS,
            "sinS": sinS,
            "masks": m,
            "onesd": ones,
        })
    return in_maps


def _run(inputs, trace=False):
    from concourse.bass_utils import run_bass_kernel_spmd

    if "nc" not in _CACHE:
        _CACHE["nc"] = _build_nc()
    nc = _CACHE["nc"]

    in_maps = _host_prep(
        inputs["x"], inputs["wq"], inputs["wk"], inputs["wv"], inputs["wo"]
    )
    res = run_bass_kernel_spmd(nc, in_maps, list(range(NCORES)), trace=trace)
    acc = None
    for c in range(NCORES):
        part = res.results[c]["outp"]
        if acc is None:
            acc = part.astype(np.float32)
        else:
            acc += part
    out = acc.reshape(B, S, D)
    return out, res


def kernel(**inputs) -> np.ndarray:
    out, _ = _run(inputs, trace=False)
    return out


# revision 16
# speedup vs baseline: 1.0015x; 1.0015x over previous
"""Multi-head causal attention (B=2, S=2048, D=2048, H=16) on 8 trn2 cores.

Sharding: tensor-parallel over heads. Core c owns heads 2c, 2c+1 (256
features of q/k/v). Each core computes its heads' QKV projections (+RoPE),
causal attention, and a partial output through its slice of wo. The 8
partial outputs are summed on the host (the "all-reduce").

All matmul operands are bf16 (fp32 PSUM accumulation) — bf16 enables the
PE fast-weight-load path and 1 col/cycle streaming; fp32 inputs are
rounded on the host. Output partials are written fp16 and summed in fp32
on the host.

Layouts on device (per core):
  qT, kT: [hd=128 partitions, head, token] bf16 (features on partitions)
          head dims permuted (evens then odds) via host-permuted wq/wk rows
          so RoPE pairs sit in partition halves.
  v:      [token, feature] bf16, computed directly with x subtiles as the
          stationary operand (lhsT) and wv as the moving operand — no PE
          transposes needed.
  scoresT chunk = matmul(lhsT=kT chunk, rhs=qT tile) -> [kt 128, q 512] PSUM
  probsT = exp(scoresT/sqrt(hd)) bf16 (no max subtraction: scores O(1))
  attnT accum = matmul(lhsT=v chunk, rhs=probsT) -> [hd, q] PSUM
  denom = matmul(lhsT=ones[128,128], rhs=probsT) -> [128, q] PSUM
          (every partition holds the denom row: broadcast for free)
  recip = reciprocal_approx_fast(denom) -> [128, q] f32
  aT    = a_ps * recip (DVE) -> bf16
  out rows = matmul(lhsT=aT t-sub, rhs=woT) -> [t 128, j] PSUM -> fp16 -> DRAM.

RoPE (per psum eviction tile qp [128, 512], top rows = even dims xr,
bottom = odd dims xi):
  qswap = [xi; xr]        (2 ACT half-copies)
  p1 = qp * [c; c]        (DVE, psum x sbuf)
  p2 = qswap * [s; -s]    (DVE)
  qT  = p1 - p2 = [xr*c - xi*s ; xi*c + xr*s]   (DVE -> bf16)
"""

import math

import numpy as np

B = 2
S = 2048
D = 2048
H = 16
HD = 128
NCORES = 8
FPC = D // NCORES          # 256 features (2 heads) per core
P = 128
ND = D // P                # 16 contraction chunks
TT_N = 512                 # token tile (matmul free dim)
NTT = S // TT_N            # 4 token tiles per batch
NKT = S // P               # 16 key chunks per batch
SCALE = 1.0 / math.sqrt(HD)

_CACHE = {}


def _build_nc():
    import concourse.bass as bass  # noqa: F401
    from concourse import bacc
    import concourse.mybir as mybir
    import concourse.tile as tile

    f32 = mybir.dt.float32
    bf16 = mybir.dt.bfloat16
    f16 = mybir.dt.float16
    MUL = mybir.AluOpType.mult
    SUB = mybir.AluOpType.subtract

    nc = bacc.Bacc(None, target_bir_lowering=False)

    xTp = nc.dram_tensor("xTp", [P, ND, B * S], bf16, kind="ExternalInput")
    wqp = nc.dram_tensor("wqp", [P, ND, FPC], bf16, kind="ExternalInput")
    wkp = nc.dram_tensor("wkp", [P, ND, FPC], bf16, kind="ExternalInput")
    wvp = nc.dram_tensor("wvp", [P, ND, FPC], bf16, kind="ExternalInput")
    wop = nc.dram_tensor("wop", [P, 2, D], bf16, kind="ExternalInput")
    cosS = nc.dram_tensor("cosS", [P, S], bf16, kind="ExternalInput")
    sinS = nc.dram_tensor("sinS", [P, S], bf16, kind="ExternalInput")  # [s; -s]
    masks = nc.dram_tensor("masks", [P, P], bf16, kind="ExternalInput")
    onesd = nc.dram_tensor("onesd", [P, P], bf16, kind="ExternalInput")
    outp = nc.dram_tensor("outp", [B * S, D], f16, kind="ExternalOutput")

    with tile.TileContext(nc) as tc:
        with (
            tc.tile_pool(name="res", bufs=1) as res,
            tc.tile_pool(name="xp", bufs=3) as xp,
            tc.tile_pool(name="aTp", bufs=2) as aTp,
            tc.tile_pool(name="probsp", bufs=8) as probsp,
            tc.tile_pool(name="ropep", bufs=2) as ropep,
            tc.tile_pool(name="recipp", bufs=2) as recipp,
            tc.tile_pool(name="ostp", bufs=4) as ostp,
            tc.tile_pool(name="ps_big", bufs=4, space="PSUM") as ps_big,
            tc.tile_pool(name="ps_av", bufs=2, space="PSUM") as ps_av,
            tc.tile_pool(name="ps_d", bufs=2, space="PSUM") as ps_d,
        ):
            # resident tensors
            wq_sb = res.tile([P, ND, FPC], bf16)
            wk_sb = res.tile([P, ND, FPC], bf16)
            wv_sb = res.tile([P, ND, FPC], bf16)
            wo_sb = res.tile([P, 2, D], bf16)
            mask_sb = res.tile([P, P], bf16)
            ones_sb = res.tile([P, P], bf16)
            qT_sbs = [res.tile([P, 2, S], bf16, name=f"qT{i}")
                      for i in range(2)]
            kT_sbs = [res.tile([P, 2, S], bf16, name=f"kT{i}")
                      for i in range(2)]
            v_sbs = [res.tile([P, NKT, FPC], bf16, name=f"v{i}")
                     for i in range(2)]
            cos_sb = res.tile([P, S], bf16)
            sin_sb = res.tile([P, S], bf16)

            # weights/tables: critical loads first (wq/wk chunk-0 halves gate
            # the first matmul; wv gates the v loop; cos/sin gate RoPE; wo
            # and ones aren't needed until attention/wo jobs)
            nc.scalar.dma_start(out=wq_sb[:, 0:8, :], in_=wqp[:, 0:8, :])
            nc.scalar.dma_start(out=wk_sb[:, 0:8, :], in_=wkp[:, 0:8, :])
            nc.scalar.dma_start(out=wq_sb[:, 8:16, :], in_=wqp[:, 8:16, :])
            nc.scalar.dma_start(out=wk_sb[:, 8:16, :], in_=wkp[:, 8:16, :])
            wo_jobs = []

            def emit_wo_job(trow0, aT, ts):
                trow = trow0 + ts * P
                ost = ostp.tile([P, D], f16, name="ost")
                for jc in range(D // TT_N):
                    o_ps = ps_big.tile([P, TT_N], f32, name="big")
                    for h in range(2):
                        nc.tensor.matmul(
                            o_ps[:],
                            aT[:, h, ts * P:(ts + 1) * P],
                            wo_sb[:, h, jc * TT_N:(jc + 1) * TT_N],
                            start=(h == 0), stop=(h == 1),
                        )
                    osl = slice(jc * TT_N, (jc + 1) * TT_N)
                    if jc % 2 == 0:
                        nc.vector.tensor_copy(ost[:, osl], o_ps[:])
                    else:
                        nc.scalar.copy(ost[:, osl], o_ps[:])
                    if jc == 1:
                        nc.sync.dma_start(
                            out=outp[trow:trow + P, 0:2 * TT_N],
                            in_=ost[:, 0:2 * TT_N])
                nc.sync.dma_start(out=outp[trow:trow + P, 2 * TT_N:D],
                                  in_=ost[:, 2 * TT_N:D])

            def emit_qkv(b, tt):
                qT_sb = qT_sbs[b % 2]
                kT_sb = kT_sbs[b % 2]
                v_sb = v_sbs[b % 2]
                t0g = b * S
                tsl = slice(tt * TT_N, (tt + 1) * TT_N)
                gsl = slice(t0g + tt * TT_N, t0g + (tt + 1) * TT_N)

                xt = xp.tile([P, ND, TT_N], bf16, name="xt")
                splits = ((0, 2, 4, 8, 16) if (b == 0 and tt == 0)
                          else (0, 4, 8, 12, 16))
                for lo, hi in zip(splits, splits[1:]):
                    nc.sync.dma_start(
                        out=xt[:, lo:hi, :],
                        in_=xTp[:, lo:hi, gsl])
                if b == 0 and tt == 0:
                    nc.sync.dma_start(out=wv_sb[:, 0:8, :], in_=wvp[:, 0:8, :])
                    nc.sync.dma_start(out=wv_sb[:, 8:16, :],
                                      in_=wvp[:, 8:16, :])
                    nc.sync.dma_start(out=cos_sb[:], in_=cosS[:])
                    nc.sync.dma_start(out=sin_sb[:], in_=sinS[:])
                    nc.sync.dma_start(out=mask_sb[:], in_=masks[:])

                qk_ps = [ps_big.tile([P, TT_N], f32, name="big")
                         for _ in range(4)]

                for d in range(ND):
                    for fc in range(2):
                        nc.tensor.matmul(
                            qk_ps[fc][:],
                            wq_sb[:, d, fc * P:(fc + 1) * P],
                            xt[:, d, :],
                            start=(d == 0), stop=(d == ND - 1),
                        )
                        nc.tensor.matmul(
                            qk_ps[2 + fc][:],
                            wk_sb[:, d, fc * P:(fc + 1) * P],
                            xt[:, d, :],
                            start=(d == 0), stop=(d == ND - 1),
                        )

                # RoPE + eviction for q,k (p2 on the idle gpsimd engine
                # keeps the DVE queue short so PSUM frees promptly)
                for i, dst in ((0, qT_sb), (1, qT_sb), (2, kT_sb), (3, kT_sb)):
                    fc = i % 2
                    qp = qk_ps[i]
                    qsw = ropep.tile([P, TT_N], f32, name="qsw")
                    nc.scalar.copy(qsw[0:64, :], qp[64:128, :])
                    nc.scalar.copy(qsw[64:128, :], qp[0:64, :])
                    p1 = ropep.tile([P, TT_N], f32, name="p1")
                    nc.vector.tensor_tensor(
                        out=p1[:], in0=qp[:], in1=cos_sb[:, tsl], op=MUL)
                    p2 = ropep.tile([P, TT_N], f32, name="p2")
                    nc.gpsimd.tensor_tensor(
                        out=p2[:], in0=qsw[:], in1=sin_sb[:, tsl], op=MUL)
                    nc.vector.tensor_tensor(
                        out=dst[:, fc, tsl], in0=p1[:], in1=p2[:], op=SUB)

                # v: [t, f] blocks, one rotating PSUM tile per t-subtile
                for sub in range(4):
                    v_ps = ps_av.tile([P, FPC], f32, name="av")
                    for d in range(ND):
                        nc.tensor.matmul(
                            v_ps[:],
                            xt[:, d, sub * P:(sub + 1) * P],
                            wv_sb[:, d, :],
                            start=(d == 0), stop=(d == ND - 1),
                        )
                    nc.scalar.copy(v_sb[:, tt * 4 + sub, :], v_ps[:])

            def emit_attention(b, qt):
                qT_sb = qT_sbs[b % 2]
                kT_sb = kT_sbs[b % 2]
                v_sb = v_sbs[b % 2]
                t0g = b * S
                nkt = 4 * qt + 4
                aT = aTp.tile([P, 2, TT_N], bf16, name="aT")
                niter = 2 * (nkt + 2)
                njobs = len(wo_jobs)
                drain_every = max(1, niter // (njobs + 1)) if njobs else 0
                it = 0
                LAG = 2
                for h in range(2):
                    a_ps = ps_av.tile([P, TT_N], f32, name="av")
                    d_ps = ps_d.tile([P, TT_N], f32, name="dp")
                    pend = {}
                    for kt in range(nkt + LAG):
                        it += 1
                        if wo_jobs and drain_every and it % drain_every == 0:
                            emit_wo_job(*wo_jobs.pop(0))
                        if kt < nkt:
                            o = kt - 4 * qt
                            c0 = max(o, 0) * P
                            csl = slice(c0, TT_N)
                            s_ps = ps_big.tile([P, TT_N], f32, name="big")
                            nc.tensor.matmul(
                                s_ps[:, csl],
                                kT_sb[:, h, kt * P:(kt + 1) * P],
                                qT_sb[:, h, qt * TT_N + c0:(qt + 1) * TT_N],
                                start=True, stop=True,
                            )
                            pr = probsp.tile([P, TT_N], bf16, name="probs")
                            nc.scalar.activation(
                                pr[:, csl], s_ps[:, csl],
                                mybir.ActivationFunctionType.Exp,
                                scale=SCALE,
                            )
                            if o >= 0:
                                nc.vector.tensor_tensor(
                                    out=pr[:, c0:c0 + P],
                                    in0=pr[:, c0:c0 + P],
                                    in1=mask_sb[:],
                                    op=MUL,
                                )
                            pend[kt] = (pr, c0)
                        j = kt - LAG
                        if j >= 0:
                            pr, c0p = pend.pop(j)
                            psl = slice(c0p, TT_N)
                            nc.tensor.matmul(
                                a_ps[:, psl],
                                v_sb[:, j, h * P:(h + 1) * P],
                                pr[:, psl],
                                start=(j == 0), stop=(j == nkt - 1),
                            )
                            nc.tensor.matmul(
                                d_ps[:, psl],
                                ones_sb[:],
                                pr[:, psl],
                                start=(j == 0), stop=(j == nkt - 1),
                            )
                            if j == nkt - 1:
                                rd = recipp.tile([P, TT_N], f32, name="rd")
                                nc.vector.reciprocal_approx_fast(
                                    out=rd[:], in_=d_ps[:])
                                nc.vector.tensor_tensor(
                                    out=aT[:, h, :], in0=a_ps[:],
                                    in1=rd[:], op=MUL)
                for ts in range(TT_N // P):
                    wo_jobs.append((t0g + qt * TT_N, aT, ts))

            # schedule: attention lags QKV by one tile, pipelined across the
            # batch boundary (batch b+1's first QKV is independent of batch
            # b's last attention)
            for b in range(B):
                emit_qkv(b, 0)
                if b == 0:
                    nc.gpsimd.dma_start(out=wo_sb[:], in_=wop[:])
                    nc.gpsimd.dma_start(out=ones_sb[:], in_=onesd[:])
                if b > 0:
                    emit_attention(b - 1, NTT - 1)
                for tt in range(1, NTT):
                    emit_qkv(b, tt)
                    emit_attention(b, tt - 1)
            emit_attention(B - 1, NTT - 1)
            while wo_jobs:
                emit_wo_job(*wo_jobs.pop(0))
    nc.compile()
    return nc


def _host_prep(x, wq, wk, wv, wo):
    import ml_dtypes

    bf16 = ml_dtypes.bfloat16

    x = np.asarray(x, dtype=np.float32)
    wq = np.asarray(wq, dtype=np.float32)
    wk = np.asarray(wk, dtype=np.float32)
    wv = np.asarray(wv, dtype=np.float32)
    wo = np.asarray(wo, dtype=np.float32)

    # x packed as [p, d, t]: xTp[p, d, t] = x[t, d*128+p]
    xTp = np.ascontiguousarray(
        x.reshape(B * S, ND, P).transpose(2, 1, 0)).astype(bf16)

    # permute q/k head dims: per head, even dims then odd dims
    perm = np.concatenate(
        [h * HD + np.concatenate([np.arange(0, HD, 2), np.arange(1, HD, 2)])
         for h in range(H)]
    )
    wq_p = wq[perm]
    wk_p = wk[perm]

    # rope tables; cos stacked twice, sin stacked [s; -s]
    inv_freq = 1.0 / (10000.0 ** (np.arange(0, HD, 2, dtype=np.float64) / HD))
    t = np.arange(S, dtype=np.float64)
    freqs = t[:, None] * inv_freq[None, :]            # [S, 64]
    cosT = np.cos(freqs).T.astype(np.float32)         # [64, S]
    sinT = np.sin(freqs).T.astype(np.float32)
    cosS = np.ascontiguousarray(np.vstack([cosT, cosT])).astype(bf16)
    sinS = np.ascontiguousarray(np.vstack([sinT, -sinT])).astype(bf16)

    # triangular causal mask for the diagonal 128x128 block
    pidx = np.arange(P)[:, None]
    qidx = np.arange(P)[None, :]
    m = np.ascontiguousarray((qidx >= pidx).astype(bf16))

    ones = np.ones((P, P), dtype=bf16)

    def pack_w(wT):  # [D, FPC] -> [128, ND, FPC]
        return np.ascontiguousarray(
            wT.reshape(ND, P, FPC).transpose(1, 0, 2)).astype(bf16)

    in_maps = []
    for c in range(NCORES):
        fs = slice(c * FPC, (c + 1) * FPC)
        woT = np.ascontiguousarray(wo[:, fs].T)        # [256, D]
        wop = np.ascontiguousarray(
            woT.reshape(2, P, D).transpose(1, 0, 2)).astype(bf16)
        in_maps.append({
            "xTp": xTp,
            "wqp": pack_w(np.ascontiguousarray(wq_p[fs].T)),
            "wkp": pack_w(np.ascontiguousarray(wk_p[fs].T)),
            "wvp": pack_w(np.ascontiguousarray(wv[fs].T)),
            "wop": wop,
            "cosS": cosS,
            "sinS": sinS,
            "masks": m,
            "onesd": ones,
        })
    return in_maps


def _run(inputs, trace=False):
    from concourse.bass_utils import run_bass_kernel_spmd

    if "nc" not in _CACHE:
        _CACHE["nc"] = _build_nc()
    nc = _CACHE["nc"]

    in_maps = _host_prep(
        inputs["x"], inputs["wq"], inputs["wk"], inputs["wv"], inputs["wo"]
    )
    res = run_bass_kernel_spmd(nc, in_maps, list(range(NCORES)), trace=trace)
    acc = None
    for c in range(NCORES):
        part = res.results[c]["outp"]
        if acc is None:
            acc = part.astype(np.float32)
        else:
            acc += part
    out = acc.reshape(B, S, D)
    return out, res


def kernel(**inputs) -> np.ndarray:
    out, _ = _run(inputs, trace=False)
    return out
